# revision 43
# baseline (speedup 1.0000x reference)
# Multi-head attention (RoPE, causal) Trainium2 Bass kernel.
#
# Sharding: 8 cores = 4 batches x 2 head-groups (8 heads each).
# Core c: batch c//2, heads (c%2)*8 .. +8. Each core computes a partial
# output projection (Wo row-parallel); host casts+sums core pairs.
#
# All storage/streaming in bf16 (x, weights, Q/K/V, P, O^T, out partials);
# matmul accumulation stays f32 in PSUM (rel err ~6.6e-3 vs the f32
# reference). x ships pre-transposed from the host ([D, L]), so there is
# no on-device transposition at all.
#
# Per-core dataflow:
#   Phases 1+2 fused, x^T resident in SBUF [128, 16, 2048] (64KB/
#   partition). Head pairs process in order 3,0,1,2; the first pair
#   interleaves its V-group / Q- / K-projection blocks with the five
#   sliced x^T loads so PE chases the DMA stream. Per pair: V natural-
#   layout (l4-outer k-inner, 4 rotating full PSUM banks), then Q^T/K^T
#   per head with fused RoPE (rotate-half via partition-offset DVE muls
#   against a partition-swapped sign-folded sin table), written straight
#   into the resident attention tiles -- all 8 heads stay in SBUF, no
#   DRAM round-trip. Weights/consts prefetch on the second (ACT) DGE
#   queue; the phase tail's last psum drains copy on DVE so the S
#   matmuls aren't gated behind one engine.
#
#   Phases 3+4 fused, block-major, software-pipelined: for each 256-wide
#   q-block j, (head, 2-tile-group) S-units are emitted DEPTH=4 ahead of
#   their PV-units so PE never head-of-line-stalls on the ACT exp
#   (~1ns/elem f32) of the group it just produced. The diagonal's odd
#   tile is causally trimmed to its live 128-wide q-half through the
#   whole S/exp/mask/PV chain (region-split accumulation stops). Causal
#   masks multiply on DVE; the denominator accumulates in bf16 on DVE
#   and one all-ones matmul per (block, head) forms the column sums;
#   accumulator PSUM banks alternate A/B between heads so PV(h+1) waits
#   on recip(h), not the full division. Block j-1's output projection is
#   flattened into single-matmul sub-ops and spread uniformly between
#   attention units (ACT keeps pace while PE stays fed); the final
#   block's last unit runs as two q-halves to shorten the drain chain.
import math

import numpy as np
import ml_dtypes

BF16 = ml_dtypes.bfloat16

P = 128
L = 2048
D = 2048
NH = 8  # heads per core
NK = D // P  # 16 contraction subtiles
NL = L // P  # 16 L chunks
NQ = 4  # q blocks (phase 2, 512 wide)
QB = 512  # phase-2 q block size
AQ = 256  # attention q block size
NAQ = L // AQ  # 8 attention blocks
SG = 2  # S-tile group size (1 PSUM bank per group, 4-deep rotation)
NPAIR = 4

_CACHE = {}


def _host_consts():
    i = np.arange(64, dtype=np.float32)
    inv_freq = (1.0 / (10000.0 ** (2.0 * i / 128.0))).astype(np.float32)
    t = np.arange(L, dtype=np.float32)
    freqs = np.outer(t, inv_freq)  # [L, 64]
    cos = np.cos(freqs).astype(np.float32)
    sin = np.sin(freqs).astype(np.float32)
    cost = np.ascontiguousarray(np.concatenate([cos, cos], axis=1).T)
    # partition-swapped sin table: rows 0:64 hold +sin (used by the bottom
    # output half), rows 64:128 hold -sin (top output half), so each RoPE
    # mul's two SBUF inputs share a base partition (DVE requirement) and
    # only the output is partition-shifted
    sint = np.ascontiguousarray(np.concatenate([sin, -sin], axis=1).T)
    ones = np.ones((P, P), np.float32)
    masks = np.zeros((2, P, AQ), np.float32)
    kk = np.arange(P)[:, None]
    qq = np.arange(AQ)[None, :]
    for tt in range(2):
        masks[tt] = (P * tt + kk <= qq).astype(np.float32)
    return (cost.astype(BF16), sint.astype(BF16), ones.astype(BF16),
            masks.astype(BF16))


def _build_module(repeats=1):
    import concourse.bacc as bacc
    import concourse.tile as tile
    import concourse.mybir as mybir
    f32 = mybir.dt.float32
    bf16 = mybir.dt.bfloat16
    Exp = mybir.ActivationFunctionType.Exp

    nc = bacc.Bacc("TRN2", target_bir_lowering=False, debug=False,
                   enable_asserts=False, num_devices=8,
                   dynamic_dma_scratch_size=2048)

    x_t = nc.dram_tensor("x", [D, L], bf16, kind="ExternalInput").ap()
    # wq/wk pre-rearranged host-side to [NH, P, NK*P]: per-head loads are
    # fully contiguous
    wq_t = nc.dram_tensor("wq", [NH, P, NK * P], bf16,
                          kind="ExternalInput").ap()
    wk_t = nc.dram_tensor("wk", [NH, P, NK * P], bf16,
                          kind="ExternalInput").ap()
    wv_t = nc.dram_tensor("wv", [D, NPAIR * 256], bf16,
                          kind="ExternalInput").ap()
    wo_t = nc.dram_tensor("wo", [NQ, P, NH, QB], bf16,
                          kind="ExternalInput").ap()
    cost_t = nc.dram_tensor("cost", [P, L], bf16, kind="ExternalInput").ap()
    sint_t = nc.dram_tensor("sint", [P, L], bf16, kind="ExternalInput").ap()
    ones_t = nc.dram_tensor("ones", [P, P], bf16, kind="ExternalInput").ap()
    masks_t = nc.dram_tensor("masks", [2, P, AQ], bf16,
                             kind="ExternalInput").ap()
    out_t = nc.dram_tensor("out", [L, D], bf16,
                           kind="ExternalOutput").ap()

    with tile.TileContext(nc) as tc:
        with tc.tile_pool(name="const", bufs=1) as const, \
             tc.tile_pool(name="dram", bufs=1, space="DRAM") as dram:
            cost = const.tile([P, L], bf16)
            sint = const.tile([P, L], bf16)
            ones = const.tile([P, P], bf16)
            maskt = const.tile([P, 2, AQ], bf16)
            # first 512 l-columns of x^T and the startup pair's V
            # weights live outside the rep scope: the next rep's loads
            # prefetch during this rep's attention, so the marginal rep
            # pays no cold-start DMA latency
            xpre = const.tile([P, NK, QB], bf16, name="xpre")
            wvpre = const.tile([P, NK, 256], bf16, name="wvpre")

            def emit_consts():
                # on the sync ring so these queue AFTER the critical pair-0
                # transposes (the Pool engine would race ahead at t=0)
                nc.scalar.dma_start(cost[:], cost_t)
                nc.scalar.dma_start(sint[:], sint_t)
                nc.scalar.dma_start(ones[:], ones_t)
                nc.scalar.dma_start(maskt[:],
                                  masks_t.rearrange("t p q -> p t q"))

            for _rep in range(repeats):
                first_rep = _rep == 0
                # attention-head tiles live from mid-phase-2 through the end
                # of attention (all 8 heads resident)
                # heads 0-5 warm-load during phase 2 (rep scope); 6-7 load
                # from a phase-3-scoped pool to fit SBUF during phase 2
                with tc.tile_pool(name="hload", bufs=1) as hload:
                    qThs, kThs, vhs = [], [], []
                    for h in range(8):
                        qThs.append(hload.tile([P, L], bf16, tag=f"qT{h}",
                                               name=f"qTh_{h}"))
                        kThs.append(hload.tile([P, L], bf16, tag=f"kT{h}",
                                               name=f"kTh_{h}"))
                        vhs.append(hload.tile([P, NL, P], bf16, tag=f"v{h}",
                                              name=f"vh_{h}"))

                    # ---- phases 1+2: x^T resident ----
                    # x arrives pre-transposed from the host ([D, L]); the
                    # resident copy loads in five straight DMAs sized so the
                    # first V-group can start after ~1MB
                    with tc.tile_pool(name="xtp", bufs=1) as xtp:
                        xt = xtp.tile([P, NK, L - QB], bf16)

                        def xt_l(lc, k):
                            # [P, 128]: lhsT chunk (V projection)
                            if lc < 4:
                                return xpre[:, k, lc * P:(lc + 1) * P]
                            return xt[:, k, lc * P - QB:(lc + 1) * P - QB]

                        def xt_r(n, k):
                            # [P, 512]: rhs l-window for proj block n
                            if n == 0:
                                return xpre[:, k, :]
                            return xt[:, k, (n - 1) * QB:n * QB]


                        # phase 2 per pair: V first, then Q^T/K^T + RoPE
                        wvr = wv_t.rearrange("(ko p) m -> p ko m", p=P)
                        with tc.tile_pool(name="p2a", bufs=1) as p2a, \
                             tc.tile_pool(name="p2b", bufs=2) as p2b, \
                             tc.tile_pool(name="ps2", bufs=1,
                                          space="PSUM") as ps2:

                            _bk = [0]

                            def pbank():
                                # rotating full-bank f32 accumulator
                                _bk[0] += 1
                                return ps2.tile([P, QB], f32, tag="bk",
                                                bufs=8,
                                                name=f"bk_{_bk[0]}")
                            def rope_spill(ps, h, ns, tgt,
                                           dve_copy=False):
                                raw = p2b.tile([P, QB], bf16,
                                               tag="raw", bufs=2)
                                if dve_copy:
                                    # phase tail: don't serialize the last
                                    # psum drains behind one engine
                                    nc.vector.tensor_copy(raw[:], ps[:])
                                else:
                                    nc.scalar.copy(raw[:], ps[:])
                                # rotate-half * sin: partition-offset muls
                                # (sint rows 64:128 hold -sin)
                                t2 = p2b.tile([P, QB], bf16, tag="t2",
                                              bufs=2)
                                nc.vector.tensor_mul(
                                    t2[0:64, :], raw[64:128, :],
                                    sint[64:128, ns])
                                nc.vector.tensor_mul(
                                    t2[64:128, :], raw[0:64, :],
                                    sint[0:64, ns])
                                t1 = p2b.tile([P, QB], bf16, tag="t1",
                                              bufs=2)
                                nc.vector.tensor_mul(
                                    t1[:], raw[:], cost[:, ns])
                                # straight into the resident
                                # attention tile
                                nc.vector.tensor_add(
                                    tgt[h][:, ns], t1[:], t2[:])

                            def load_wts(wr, pr_, q=None):
                                q = q or nc.scalar
                                wts = []
                                for hh in (2 * pr_, 2 * pr_ + 1):
                                    wt = p2b.tile([P, NK, P], bf16,
                                                  tag=f"w{hh % 2}",
                                                  name=f"wt_{hh % 2}",
                                                  bufs=2)
                                    q.dma_start(
                                        wt.rearrange("p k m -> p (k m)"),
                                        wr[hh])
                                    wts.append(wt)
                                return wts

                            def v_group(lg, wvp, pr_):
                                # 4-column V pass in 4 rotating PSUM banks;
                                # l4-outer so the startup stream can begin
                                # after a single transposed chunk
                                pss = [pbank() for _ in range(4)]
                                for l4 in range(4):
                                    lc = lg * 4 + l4
                                    for k in range(NK):
                                        nc.tensor.matmul(
                                            pss[l4][:, 0:256],
                                            lhsT=xt_l(lc, k),
                                            rhs=wvp[:, k, :],
                                            start=(k == 0),
                                            stop=(k == NK - 1))
                                for l4 in range(4):
                                    lc = lg * 4 + l4
                                    for hp in range(2):
                                        src = pss[l4][:,
                                                      hp * P:(hp + 1) * P]
                                        dst = vhs[2 * pr_ + hp][:, lc, :]
                                        if (lc + hp) % 2 == 0:
                                            nc.vector.tensor_copy(dst, src)
                                        else:
                                            nc.scalar.copy(dst, src)

                            def proj_block(wts, tgt, pr_, n,
                                           last=False):
                                ns = slice(n * QB, (n + 1) * QB)
                                pss = [pbank() for _ in range(2)]
                                # both heads share rhs per k
                                for k in range(NK):
                                    for d_ in range(2):
                                        nc.tensor.matmul(
                                            pss[d_][:],
                                            lhsT=wts[d_][:, k, :],
                                            rhs=xt_r(n, k),
                                            start=(k == 0),
                                            stop=(k == NK - 1))
                                for d_ in range(2):
                                    rope_spill(pss[d_], 2 * pr_ + d_, ns,
                                               tgt, dve_copy=last
                                               and d_ == 1)

                            # pair 3 (the only spill pair) goes first and
                            # carries the transpose-interleaved startup; the
                            # last pair then ends on direct SBUF writes with
                            # no DMA tail before the phase barrier
                            for idx, pr in enumerate((3, 0, 1, 2)):
                                startup = idx == 0
                                if startup:
                                    wvp = wvpre
                                else:
                                    wvp = p2a.tile([P, NK, 256], bf16,
                                                   tag="wvp", bufs=2)
                                    nc.scalar.dma_start(
                                        wvp[:],
                                        wvr[:, :, pr * 256:(pr + 1) * 256])
                                if startup:
                                    # x^T loads in l-slices: the first two
                                    # 1MB slices release V-group 0 early,
                                    # then 2MB quarters stream ahead of the
                                    # interleaved V/Q/K blocks
                                    xTr = x_t.rearrange(
                                        "(ko p) l -> p ko l", p=P)

                                    def xload(l0, l1):
                                        if l1 <= QB:
                                            nc.sync.dma_start(
                                                xpre[:, :, l0:l1],
                                                xTr[:, :, l0:l1])
                                        else:
                                            nc.sync.dma_start(
                                                xt[:, :, l0 - QB:l1 - QB],
                                                xTr[:, :, l0:l1])

                                    nc.scalar.dma_start(
                                        wvpre[:],
                                        wvr[:, :, pr * 256:(pr + 1) * 256])
                                    xload(0, 256)
                                    xload(256, 512)
                                    wtq = load_wts(wq_t, pr)
                                    wtk = load_wts(wk_t, pr)
                                    xload(512, 768)
                                    xload(768, 1024)
                                    v_group(0, wvp, pr)
                                    xload(1024, 1280)
                                    xload(1280, 1536)
                                    if first_rep:
                                        emit_consts()
                                    proj_block(wtq, qThs, pr, 0)
                                    xload(1536, 1792)
                                    xload(1792, 2048)
                                    v_group(1, wvp, pr)
                                    proj_block(wtk, kThs, pr, 0)
                                    v_group(2, wvp, pr)
                                    proj_block(wtq, qThs, pr, 1)
                                    v_group(3, wvp, pr)
                                    for n in range(1, NQ):
                                        proj_block(wtk, kThs, pr, n)
                                    for n in range(2, NQ):
                                        proj_block(wtq, qThs, pr, n)
                                else:
                                    for lg in range(4):
                                        v_group(lg, wvp, pr)
                                if not startup:
                                    for wr, tgt in ((wq_t, qThs),
                                                    (wk_t, kThs)):
                                        wts = load_wts(wr, pr)
                                        for n in range(NQ):
                                            proj_block(
                                                wts, tgt, pr, n,
                                                last=pr == 2
                                                and wr is wk_t
                                                and n >= NQ - 2)


                    # ---- phases 3+4 fused, block-major ----
                    with tc.tile_pool(name="otp", bufs=1) as otp, \
                         tc.tile_pool(name="p3t", bufs=2) as p3t, \
                         tc.tile_pool(name="ps3", bufs=1, space="PSUM") as ps3, \
                         tc.tile_pool(name="p4", bufs=1) as p4:
                        oTs = []
                        for h in range(NH):
                            oTs.append(otp.tile([P, L], bf16, tag=f"oT{h}",
                                                name=f"oT_{h}"))
                        wo_sb = otp.tile([P, NQ, NH, QB], bf16, tag="wo")
                        outr = out_t.rearrange("(lo p) n -> p lo n", p=P)

                        class OutProj:
                            """Block j's output projection, emitted as
                            fine-grained sub-ops (one h-accumulation matmul
                            or one copy+DMA each) so it can be spread
                            uniformly between attention units."""
                            def __init__(self, j):
                                self.j = j
                                self.sub = [(u, h) for u in range(2 * NQ)
                                            for h in range(NH + 1)]
                                self.pos = 0
                                self.pp = None

                            def remaining(self):
                                return len(self.sub) - self.pos

                            def emit_one(self):
                                u, h = self.sub[self.pos]
                                self.pos += 1
                                lc = 2 * self.j + u // NQ
                                n = u % NQ
                                ns = slice(n * QB, (n + 1) * QB)
                                if h == 0:
                                    self.pp = ps3.tile([P, QB], f32,
                                                       tag="op4", bufs=2,
                                                       name="op4_t")
                                if h < NH:
                                    nc.tensor.matmul(
                                        self.pp[:],
                                        lhsT=oTs[h][:, lc * P:(lc + 1) * P],
                                        rhs=wo_sb[:, n, h, :],
                                        start=(h == 0), stop=(h == NH - 1))
                                    return
                                osb = p4.tile([P, QB], bf16, tag="osb",
                                              bufs=4)
                                if (lc + n) % 2 == 0:
                                    nc.vector.tensor_copy(osb[:], self.pp[:])
                                else:
                                    nc.scalar.copy(osb[:], self.pp[:])
                                nc.sync.dma_start(outr[:, lc, ns], osb[:])

                        # Wo prefetch per n-block: unit n of out_proj(0)
                        # only needs block n, so the first units unblock
                        # after one 1MB transfer
                        for n_ in range(NQ):
                            nc.sync.dma_start(wo_sb[:, n_, :, :],
                                              wo_t[n_])
                        # Per-block software pipeline: each (head, group)
                        # S-unit is emitted one step ahead of its PV-unit,
                        # so PE doesn't head-of-line-stall on the exp (ACT)
                        # of the group it just produced. Block j-1's output
                        # projection units interleave at head boundaries.
                        def emit_s(j, h, gi, nt, state):
                            # last group holds the two diagonal tiles; its
                            # second (odd) tile only lives on the upper
                            # q-half, so S/exp/mask/PV run 128-wide there
                            qs = slice(j * AQ, (j + 1) * AQ)
                            g0 = gi * SG
                            gs = min(SG, nt - g0)
                            last = g0 + gs - 1 == 2 * j + 1
                            sp = ps3.tile([P, SG * AQ], f32, tag="spg",
                                          bufs=4)
                            take = gs * AQ
                            for d_ in range(gs):
                                i = g0 + d_
                                if last and i == 2 * j + 1:
                                    take = d_ * AQ + P
                                    nc.tensor.matmul(
                                        sp[:, d_ * AQ:d_ * AQ + P],
                                        lhsT=state["kTh"][
                                            :, i * P:(i + 1) * P],
                                        rhs=state["qTh"][
                                            :, j * AQ + P:(j + 1) * AQ],
                                        start=True, stop=True)
                                else:
                                    nc.tensor.matmul(
                                        sp[:, d_ * AQ:(d_ + 1) * AQ],
                                        lhsT=state["kTh"][
                                            :, i * P:(i + 1) * P],
                                        rhs=state["qTh"][:, qs],
                                        start=True, stop=True)
                            pt = p3t.tile([P, SG * AQ], bf16, tag="pt",
                                          bufs=8)
                            nc.scalar.activation(pt[:, :take],
                                                 sp[:, :take], Exp)
                            for d_ in range(gs):
                                i = g0 + d_
                                if i == 2 * j:
                                    nc.vector.tensor_mul(
                                        pt[:, d_ * AQ:(d_ + 1) * AQ],
                                        pt[:, d_ * AQ:(d_ + 1) * AQ],
                                        maskt[:, 0, :])
                                elif i == 2 * j + 1:
                                    nc.vector.tensor_mul(
                                        pt[:, d_ * AQ:d_ * AQ + P],
                                        pt[:, d_ * AQ:d_ * AQ + P],
                                        maskt[:, 0, 0:P])
                            # denominator accumulation on DVE
                            for d_ in range(gs):
                                i = g0 + d_
                                seg = pt[:, d_ * AQ:(d_ + 1) * AQ]
                                if gi == 0 and d_ == 0:
                                    bacc_t = p3t.tile([P, AQ], bf16,
                                                      tag="bacc", bufs=2)
                                    state["bacc"] = bacc_t[:]
                                    if last and i == 2 * j:
                                        # next tile is the trimmed half:
                                        # start from a copy
                                        nc.vector.tensor_copy(
                                            state["bacc"], seg)
                                    # else: d_=1 seeds bacc = pt0 + pt1
                                    continue
                                if last and i == 2 * j + 1:
                                    nc.vector.tensor_add(
                                        state["bacc"][:, P:AQ],
                                        state["bacc"][:, P:AQ],
                                        pt[:, d_ * AQ:d_ * AQ + P])
                                elif gi == 0 and d_ == 1:
                                    nc.vector.tensor_add(
                                        state["bacc"], pt[:, 0:AQ], seg)
                                else:
                                    nc.vector.tensor_add(
                                        state["bacc"], state["bacc"], seg)
                            return (pt, g0, gs, last)

                        def emit_pv(j, nt, state, su):
                            pt, g0, gs, last = su
                            for d_ in range(gs):
                                i = g0 + d_
                                if last and i == 2 * j:
                                    # split so each q-half region gets its
                                    # own accumulation stop
                                    nc.tensor.matmul(
                                        state["ops"][:, 0:P],
                                        lhsT=state["vh"][:, i, :],
                                        rhs=pt[:, d_ * AQ:d_ * AQ + P],
                                        start=(i == 0), stop=True)
                                    nc.tensor.matmul(
                                        state["ops"][:, P:AQ],
                                        lhsT=state["vh"][:, i, :],
                                        rhs=pt[:, d_ * AQ + P:
                                               (d_ + 1) * AQ],
                                        start=(i == 0), stop=False)
                                elif last and i == 2 * j + 1:
                                    nc.tensor.matmul(
                                        state["ops"][:, P:AQ],
                                        lhsT=state["vh"][:, i, :],
                                        rhs=pt[:, d_ * AQ:d_ * AQ + P],
                                        start=False, stop=True)
                                else:
                                    nc.tensor.matmul(
                                        state["ops"],
                                        lhsT=state["vh"][:, i, :],
                                        rhs=pt[:, d_ * AQ:(d_ + 1) * AQ],
                                        start=(i == 0), stop=False)

                        def emit_head_close(j, h, state):
                            qs = slice(j * AQ, (j + 1) * AQ)
                            sums_t = ps3.tile([P, AQ], f32,
                                              tag=state["sumtag"], bufs=1,
                                              name="sums_t")
                            nc.tensor.matmul(sums_t[:], lhsT=ones[:],
                                             rhs=state["bacc"], start=True,
                                             stop=True)
                            rec = p3t.tile([P, AQ], f32, tag="rec", bufs=2)
                            nc.vector.reciprocal(rec[:], sums_t[:])
                            nc.vector.tensor_mul(oTs[h][:, qs],
                                                 state["ops"], rec[:])

                        from collections import deque
                        DEPTH = 4
                        for j in range(NAQ):
                            nt = 2 * j + 2
                            ngrp = (nt + SG - 1) // SG
                            steps = NH * ngrp
                            opq = OutProj(j - 1) if j > 0 else None
                            states = {}
                            pend = deque()

                            def do_pv(ph, pgi, su):
                                emit_pv(j, nt, states[ph], su)
                                if pgi == ngrp - 1:
                                    emit_head_close(j, ph, states[ph])

                            step = 0
                            for h in range(NH):
                                for gi in range(ngrp):
                                    if gi == 0:
                                        # accumulator banks alternate A/B
                                        # between heads: PV(h+1,0) then
                                        # waits only on recip(h) (which
                                        # frees h's sums bank), not on the
                                        # slower ops-read mul(h)
                                        ta, tb = (("accA", "accB")
                                                  if h % 2 == 0 else
                                                  ("accB", "accA"))
                                        ops_t = ps3.tile(
                                            [P, AQ], f32, tag=ta,
                                            bufs=1, name="ops_t")[:]
                                        states[h] = {
                                            "qTh": qThs[h], "kTh": kThs[h],
                                            "vh": vhs[h], "ops": ops_t,
                                            "sumtag": tb,
                                        }
                                    su = emit_s(j, h, gi, nt, states[h])
                                    if len(pend) >= DEPTH:
                                        do_pv(*pend.popleft())
                                    pend.append((h, gi, su))
                                    step += 1
                                    if opq is not None:
                                        total = len(opq.sub)
                                        tgt = (total * step) // steps
                                        while opq.pos < tgt:
                                            opq.emit_one()
                            while pend:
                                do_pv(*pend.popleft())
                            if opq is not None:
                                while opq.remaining():
                                    opq.emit_one()
                        tailp = OutProj(NAQ - 1)
                        tailp.sub = tailp.sub[:-(NH + 1)]
                        while tailp.remaining():
                            tailp.emit_one()
                        # last unit in two halves: the final copy+DMA chain
                        # is half as long
                        lc_, n_ = 2 * (NAQ - 1) + 1, NQ - 1
                        ns_ = slice(n_ * QB, (n_ + 1) * QB)
                        pp_ = ps3.tile([P, QB], f32, tag="op4", bufs=2,
                                       name="op4_t")
                        for half in range(2):
                            hs = slice(half * AQ, (half + 1) * AQ)
                            for h in range(NH):
                                nc.tensor.matmul(
                                    pp_[:, hs],
                                    lhsT=oTs[h][:, lc_ * P:(lc_ + 1) * P],
                                    rhs=wo_sb[:, n_, h, hs],
                                    start=(h == 0), stop=(h == NH - 1))
                            osb_ = p4.tile([P, AQ], bf16, tag="osbh",
                                           bufs=2)
                            if half == 0:
                                nc.vector.tensor_copy(osb_[:], pp_[:, hs])
                            else:
                                nc.scalar.copy(osb_[:], pp_[:, hs])
                            nc.sync.dma_start(
                                outr[:, lc_,
                                     n_ * QB + half * AQ:
                                     n_ * QB + (half + 1) * AQ],
                                osb_[:])

    nc.compile()
    return nc


def get_module(repeats=1):
    key = ("nc", repeats)
    if key not in _CACHE:
        _CACHE[key] = _build_module(repeats)
    return _CACHE[key]


def make_in_maps(x, Wq, Wk, Wv, Wo):
    cost, sint, ones, masks = _host_consts()
    s = np.float32(1.0 / math.sqrt(128.0))
    in_maps = []
    for c in range(8):
        b, hg = divmod(c, 2)
        cs = slice(hg * 1024, (hg + 1) * 1024)

        def _wqk(W, scale=np.float32(1.0)):
            # [D, 1024] -> [NH, P, NK*P]: per-head loads fully contiguous
            w4 = (np.asarray(W[:, cs], np.float32) * scale).reshape(
                NK, P, NH, P)
            return np.ascontiguousarray(
                w4.transpose(2, 1, 0, 3).reshape(NH, P, NK * P)).astype(BF16)

        in_maps.append({
            "x": np.ascontiguousarray(
                np.asarray(x[b], np.float32).T).astype(BF16),
            "wq": _wqk(Wq, s),
            "wk": _wqk(Wk),
            "wv": np.ascontiguousarray(Wv[:, cs]).astype(BF16),
            "wo": np.ascontiguousarray(
                np.asarray(Wo[cs, :], np.float32).reshape(
                    NH, P, NQ, QB).transpose(2, 1, 0, 3)).astype(BF16),
            "cost": cost, "sint": sint, "ones": ones,
            "masks": masks,
        })
    return in_maps


def kernel(x, Wq, Wk, Wv, Wo):
    from concourse import bass_utils

    nc = get_module()
    in_maps = make_in_maps(x, Wq, Wk, Wv, Wo)
    res = bass_utils.run_bass_kernel_spmd(nc, in_maps, core_ids=list(range(8)))
    outs = [r["out"] for r in res.results]
    out = np.empty((4, L, D), np.float32)
    for b in range(4):
        out[b] = (outs[2 * b].astype(np.float32)
                  + outs[2 * b + 1].astype(np.float32))
    return out

